# revision 3
# baseline (speedup 1.0000x reference)
"""Causal MHA block (B=4,S=2048,E=1024,H=16,D=64) on 8 trn2 cores — v4.

Sharding: 4 batches x 2 head-groups (8 heads each) = 8 cores; host sums the
2 partials per batch and adds b_out.

Layout (cost model charges a matmul out_free_size rows regardless of
contraction depth or output partition count):
  - AV in NATURAL layout: ans[q=128, 65] per (head, q-subtile); lhsT =
    exp'd scores w[k, q] (stationary), rhs = V[k, 65] (64 v-cols + ones col
    -> fused softmax denominator). Dead q-subtiles above the diagonal are
    skipped; diagonal subtile masked with a [128,128] triangle.
  - All inputs bf16 (1 cyc/row at ANY width -> exact causal widths); QKV
    projection matmuls bf16 with f32 psum accumulation.
  - ans normalized per q-subtile as soon as its last AV lands (reciprocal
    of the denom col + one strided stt-mul), PE-transposed (bf16 identity)
    into the ans psum banks (bitcast), copied to SBUF for the out proj.

Schedule: attention(qb) is ACT(exp)-bound and increasingly so for later
blocks; projections/out-projections are PE-only. Each s-block's projection
is split into a prefix (x/w loads, Q all, K ft0, V all) that gates
attention(qb), and a postfix (K ft1-3) consumed as filler inside
attention(qb). Out-projections are deferred into the latest (most
ACT-bound) blocks. Fillers are drip-fed with per-generator rate/delay.
"""

import numpy as np

B, S, E, H, D = 4, 2048, 1024, 16, 64
NCORES = 8
HG = 2                 # head groups (tensor parallel)
HC = H // HG           # 8 heads per core
FQ = HC * D            # 512 local features per q/k/v
P, NB = 128, 512       # partition tile, free-dim block
ET, ST, KTN, FT = E // P, S // NB, S // P, FQ // P   # 8, 4, 16, 4
EP = ET // 2           # e pair-tiles
DV = D + 1             # v cols + ones (denominator) col

_cache = {}


def _build():
    from contextlib import ExitStack
    import concourse.tile as tile
    import concourse.mybir as mybir
    from concourse import bacc

    dt = mybir.dt
    f32, bf16 = dt.float32, dt.bfloat16
    AF = mybir.ActivationFunctionType
    ALU = mybir.AluOpType
    SCALE = 0.125  # 1/sqrt(D)

    nc = bacc.Bacc("TRN2", target_bir_lowering=False, debug=False,
                   num_devices=NCORES)

    xT = nc.dram_tensor("xT", [E, S], bf16, kind="ExternalInput").ap()
    wq = nc.dram_tensor("wq", [E, FQ], bf16, kind="ExternalInput").ap()
    wk = nc.dram_tensor("wk", [E, FQ], bf16, kind="ExternalInput").ap()
    wv = nc.dram_tensor("wv", [E, FQ], bf16, kind="ExternalInput").ap()
    wo = nc.dram_tensor("wo", [FQ, E], bf16, kind="ExternalInput").ap()
    msk = nc.dram_tensor("msk", [P, P], bf16, kind="ExternalInput").ap()
    idn = nc.dram_tensor("idn", [P, P], bf16, kind="ExternalInput").ap()
    bq = nc.dram_tensor("bq", [FQ], f32, kind="ExternalInput").ap()
    bk = nc.dram_tensor("bk", [FQ], f32, kind="ExternalInput").ap()
    bvb = nc.dram_tensor("bvb", [P, FQ], f32, kind="ExternalInput").ap()
    outT = nc.dram_tensor("outT", [E, S], f32, kind="ExternalOutput").ap()

    with tile.TileContext(nc) as tc:
        with ExitStack() as ctx:
            pers = ctx.enter_context(tc.tile_pool(name="pers", bufs=1))
            pmisc = ctx.enter_context(tc.tile_pool(name="pmisc", bufs=1))
            px = ctx.enter_context(tc.tile_pool(name="px", bufs=2))
            pwk = ctx.enter_context(tc.tile_pool(name="pwk", bufs=2))
            pwv = ctx.enter_context(tc.tile_pool(name="pwv", bufs=1))
            pwq = ctx.enter_context(tc.tile_pool(name="pwq", bufs=1))
            pqts = ctx.enter_context(tc.tile_pool(name="pqts", bufs=2))
            pwe = ctx.enter_context(tc.tile_pool(name="pwe", bufs=5))
            pans = ctx.enter_context(tc.tile_pool(name="pans", bufs=2))
            patt = ctx.enter_context(tc.tile_pool(name="patt", bufs=2))
            pepi = ctx.enter_context(tc.tile_pool(name="pepi", bufs=3))
            pout = ctx.enter_context(tc.tile_pool(name="pout", bufs=4))
            ps1 = ctx.enter_context(
                tc.tile_pool(name="ps1", bufs=2, space="PSUM"))
            sps = ctx.enter_context(
                tc.tile_pool(name="sps", bufs=2, space="PSUM"))
            avps = ctx.enter_context(
                tc.tile_pool(name="avps", bufs=1, space="PSUM"))

            KT = [pers.tile([P, S], bf16, tag=f"kt{i}", name=f"kt{i}")
                  for i in range(FT)]
            Vp = [pers.tile([P, HC * DV], bf16, tag=f"vp{i}",
                            name=f"vp{i}") for i in range(KTN)]
            bqt = pers.tile([P, FT], f32, tag="bqt")
            bkt = pers.tile([P, FT], f32, tag="bkt")
            bvt = pers.tile([P, FQ], f32, tag="bvt")

            mtri = pmisc.tile([P, P], bf16, tag="mtri")
            ident = pmisc.tile([P, P], bf16, tag="ident")
            wouts = [pmisc.tile([P, E], bf16, tag=f"wo{ft}", name=f"wo{ft}")
                     for ft in range(FT)]

            def late_loads_gen():
                """Non-critical loads kept off the startup critical path."""
                nc.gpsimd.dma_start(bqt[:], bq.rearrange("(a p) -> p a", p=P))
                nc.gpsimd.dma_start(bkt[:], bk.rearrange("(a p) -> p a", p=P))
                nc.gpsimd.dma_start(bvt[:], bvb[:])
                nc.gpsimd.dma_start(mtri[:], msk[:])
                nc.gpsimd.dma_start(ident[:], idn[:])
                yield

            def wout_gen():
                """wout loads; needed only by OUT(0), driven as a filler so
                they stay off the startup critical path."""
                for ft in range(FT):
                    eng = nc.scalar if ft % 2 else nc.sync
                    eng.dma_start(wouts[ft][:],
                                  wo[ft * P:(ft + 1) * P, :])
                    yield

            # per-block state shared between generators
            QTS = {}    # sb -> [4 tiles]
            XTS = {}    # sb -> [4 pair tiles [P, 2, NB]]
            KWT = {}    # sb -> [4 wk pair tiles]
            ATS = {}    # qb -> at tile [P, 4*NB] bf16 (q-part, qsub, hd)
            ATT = {}    # qb -> [4 atT tiles [P, NB] bf16] (hd-part, q)
            WQ = []     # resident wq pair tiles

            def ppre_gen(sb):
                """Projection prefix of s-block sb: x/w loads, Q (all ft),
                K (ft0), V (all). Gates attention(sb)."""
                xts = []
                for ep in range(EP):
                    t = px.tile([P, 2, NB], bf16, tag=f"x{ep}",
                                name=f"x{ep}_{sb}")
                    nc.sync.dma_start(
                        t[:], xT[2 * ep * P:(2 * ep + 2) * P,
                                 sb * NB:(sb + 1) * NB]
                        .rearrange("(a p) s -> p a s", p=P))
                    xts.append(t)
                    if sb == 0:
                        wt = pwq.tile([P, 2, FQ], bf16, tag=f"wq{ep}",
                                      name=f"wq{ep}")
                        WQ.append(wt)
                        # interleave wq with x so pairs arrive together
                        nc.scalar.dma_start(
                            wt[:], wq[2 * ep * P:(2 * ep + 2) * P, :]
                            .rearrange("(a p) f -> p a f", p=P))
                XTS[sb] = xts
                kwts = []
                for ep in range(EP):
                    t = pwk.tile([P, 2, FQ], bf16, tag=f"wk{ep}",
                                name=f"wk{ep}_{sb}")
                    nc.scalar.dma_start(
                        t[:], wk[2 * ep * P:(2 * ep + 2) * P, :]
                        .rearrange("(a p) f -> p a f", p=P))
                    kwts.append(t)
                KWT[sb] = kwts
                vwts = []
                for ep in range(EP):
                    t = pwv.tile([P, 2, FQ], bf16, tag=f"wv{ep}",
                                name=f"wv{ep}_{sb}")
                    nc.sync.dma_start(
                        t[:], wv[2 * ep * P:(2 * ep + 2) * P, :]
                        .rearrange("(a p) f -> p a f", p=P))
                    vwts.append(t)
                yield
                # Q pass (all ft)
                QTS[sb] = []
                for ft in range(FT):
                    ps = ps1.tile([P, NB], f32, tag="ps", name=f"psq{ft}_{sb}")
                    for e in range(ET):
                        nc.tensor.matmul(
                            ps[:], WQ[e // 2][:, e % 2, ft * P:(ft + 1) * P],
                            xts[e // 2][:, e % 2, :], start=(e == 0),
                            stop=(e == ET - 1))
                        if e == 3:
                            yield
                    qt = pqts.tile([P, NB], bf16, tag=f"qts{ft}",
                                   name=f"qts{ft}_{sb}")
                    nc.vector.tensor_scalar_add(qt[:], ps[:],
                                                bqt[:, ft:ft + 1])
                    QTS[sb].append(qt)
                    yield
                # K pass ft0 only (needed by hp0's diagonal)
                yield from kpass(sb, 0)
                # V pass (all; needed by hp0's diagonal AVs)
                for stl in range(ST):
                    st = ST * sb + stl
                    ps = ps1.tile([P, NB], f32, tag="ps",
                                  name=f"psv{stl}_{sb}")
                    for e in range(ET):
                        nc.tensor.matmul(
                            ps[:], xts[e // 2][:, e % 2, stl * P:(stl + 1) * P],
                            vwts[e // 2][:, e % 2, :], start=(e == 0),
                            stop=(e == ET - 1))
                        if e == 3:
                            yield
                    vview = Vp[st][:].rearrange("p (h c) -> p h c", c=DV)
                    nc.vector.memset(vview[:, :, D:DV], 1.0)
                    nc.vector.scalar_tensor_tensor(
                        vview[:, :, 0:D], ps[:], 1.0,
                        bvt[:].rearrange("p (h d) -> p h d", d=D),
                        op0=ALU.mult, op1=ALU.add)
                    yield

            def kpass(sb, ft):
                ps = ps1.tile([P, NB], f32, tag="ps", name=f"psk{ft}_{sb}")
                for e in range(ET):
                    nc.tensor.matmul(
                        ps[:], KWT[sb][e // 2][:, e % 2, ft * P:(ft + 1) * P],
                        XTS[sb][e // 2][:, e % 2, :], start=(e == 0),
                        stop=(e == ET - 1))
                    if e == 3:
                        yield
                nc.vector.tensor_scalar_add(
                    KT[ft][:, sb * NB:(sb + 1) * NB], ps[:],
                    bkt[:, ft:ft + 1])
                yield

            def ppost_gen(sb):
                """Projection postfix: K ft1-3, consumed inside
                attention(sb) before each hp reaches its diagonal."""
                for ft in range(1, FT):
                    yield from kpass(sb, ft)

            def attn_gen(qb):
                """Attention for q-block qb. Yields once per kt step.
                The per-qsub epilogue (normalize) fires as soon as that
                q-subtile's last AV lands; each head-pair's at columns are
                PE-transposed right after its last epilogue."""
                nkt = ST * (qb + 1)
                QTs = QTS[qb]
                at = pans.tile([P, ST * NB], bf16, tag="at", name=f"at{qb}")
                ATS[qb] = at
                ATT[qb] = [None] * FT
                atv = at[:].rearrange("p (g f) -> p g f", f=NB)
                for hp in range(FT):
                    # ans psum: per head i: [P, 4*DV] = 4 q-subtiles x
                    # (64 v + denom); accumulation groups per slice.
                    ans = [avps.tile([P, ST * DV], f32, tag=f"an{i}",
                                     name=f"an{qb}_{hp}_{i}")
                           for i in range(2)]
                    anvs = [a[:].rearrange("p (g c) -> p g c", c=DV)
                            for a in ans]
                    recs = [pepi.tile([P, ST], f32, tag=f"rec{i}",
                                      name=f"rec{qb}_{hp}_{i}")
                            for i in range(2)]

                    def emit_epi(g):
                        # normalize q-subtile g of both heads: its last AV
                        # (kt = ST*qb + g) has landed.
                        for i in range(2):
                            nc.vector.reciprocal(
                                recs[i][:, g:g + 1]
                                .rearrange("p (a c) -> p a c", c=1),
                                anvs[i][:, g:g + 1, D:DV])
                            nc.vector.tensor_scalar_mul(
                                atv[:, g:g + 1,
                                    (2 * hp + i) * D:(2 * hp + i + 1) * D],
                                anvs[i][:, g:g + 1, 0:D],
                                recs[i][:, g:g + 1])

                    def emit_av(ent, last):
                        kt0, j0, w0 = ent
                        wvw = w0[:].rearrange("p (h q) -> p h q", h=2)
                        for i in range(2):
                            rv = Vp[kt0][:, (2 * hp + i) * DV:
                                         (2 * hp + i + 1) * DV]
                            for g in range(max(j0, 0), ST):
                                # start=True lazily zeroes the WHOLE 2KB
                                # psum bank (ZERO_REGION), so only the first
                                # matmul into the bank may set it; later
                                # slices accumulate onto the zeroed region.
                                nc.tensor.matmul(
                                    ans[i][:, g * DV:(g + 1) * DV],
                                    wvw[:, i, g * P:(g + 1) * P],
                                    rv,
                                    start=(kt0 == 0 and g == max(j0, 0)),
                                    stop=(kt0 == ST * qb + g),
                                    skip_group_check=True)
                        if j0 >= 0:
                            emit_epi(j0)

                    pend = []
                    for kt in range(nkt):
                        j = kt - ST * qb
                        c0 = j * P if j >= 0 else 0
                        # both heads of the pair share one 2-bank psum tile
                        # and a single strided exp call
                        sp = sps.tile([P, 2 * NB], f32, tag="sp",
                                      name=f"sp{qb}_{hp}_{kt}")
                        for i in range(2):
                            nc.tensor.matmul(
                                sp[:, i * NB + c0:(i + 1) * NB],
                                KT[hp][i * D:(i + 1) * D,
                                       kt * P:(kt + 1) * P],
                                QTs[hp][i * D:(i + 1) * D, c0:NB],
                                start=True, stop=True)
                        w = pwe.tile([P, 2 * NB], bf16, tag="w",
                                     name=f"w{qb}_{hp}_{kt}")
                        spv = sp[:].rearrange("p (h q) -> p h q", h=2)
                        wv_ = w[:].rearrange("p (h q) -> p h q", h=2)
                        nc.scalar.activation(wv_[:, :, c0:NB],
                                             spv[:, :, c0:NB],
                                             AF.Exp, scale=SCALE)
                        if j >= 0:
                            nc.vector.tensor_mul(
                                wv_[:, :, c0:c0 + P], wv_[:, :, c0:c0 + P],
                                mtri[:]
                                .rearrange("p (a q) -> p a q", a=1)
                                .broadcast_to([P, 2, P]))
                        pend.append((kt, j, w))
                        if len(pend) > 2:
                            emit_av(pend.pop(0), last=False)
                        yield
                    while pend:
                        ent = pend.pop(0)
                        emit_av(ent, last=not pend)
                        yield
                    # transpose this head-pair's at columns into atT
                    # (the ans psum banks are free again; bitcast bf16)
                    tpf = avps.tile([P, ST * DV], f32, tag=f"an{hp % 2}",
                                    name=f"tp{qb}_{hp}")
                    tp = tpf[:].bitcast(bf16)
                    for g in range(ST):
                        # one bank-zeroing group across the 4 block writes
                        nc.tensor.matmul(
                            tp[:, g * P:(g + 1) * P],
                            atv[:, g, hp * P:(hp + 1) * P],
                            ident[:], is_transpose=True,
                            start=(g == 0), stop=(g == ST - 1),
                            skip_group_check=True)
                    yield
                    att = patt.tile([P, NB], bf16, tag=f"att{hp}",
                                    name=f"att{qb}_{hp}")
                    nc.vector.tensor_copy(att[:], tp[:, 0:NB])
                    ATT[qb][hp] = att
                    yield

            def out_gen(qb):
                """Output projection of q-block qb. Yields per e-tile."""
                for et in range(ET):
                    po = ps1.tile([P, NB], f32, tag="ps", name=f"po{qb}_{et}")
                    for ft in range(FT):
                        nc.tensor.matmul(po[:],
                                         wouts[ft][:, et * P:(et + 1) * P],
                                         ATT[qb][ft][:], start=(ft == 0),
                                         stop=(ft == FT - 1))
                    ott = pout.tile([P, NB], f32, tag="ot",
                                    name=f"ot{qb}_{et}")
                    if qb == ST - 1 and et % 2:
                        nc.scalar.copy(ott[:], po[:])
                    else:
                        nc.vector.tensor_copy(ott[:], po[:])
                    if qb == ST - 1:
                        eng = (nc.sync, nc.scalar)[et % 2]
                    else:
                        eng = nc.gpsimd
                    eng.dma_start(
                        outT[et * P:(et + 1) * P, qb * NB:(qb + 1) * NB],
                        ott[:])
                    yield

            def drain(g):
                for _ in g:
                    pass

            gens = {}

            def gget(key, mk):
                if key not in gens:
                    gens[key] = mk()
                return gens[key]

            p0 = gget(("pre", 0), lambda: ppre_gen(0))
            next(p0)          # x/w DMAs emitted first
            drain(late_loads_gen())
            drain(p0)
            # Filler plan per attention block: (key, maker, rate, delay).
            # Rates in filler-units per attention kt-step; delay in steps.
            plans = {
                0: [(("post", 0), lambda: ppost_gen(0), 0.5, 0),
                    (("wout",), wout_gen, 0.3, 2),
                    (("pre", 1), lambda: ppre_gen(1), 1.0, 6)],
                1: [(("post", 1), lambda: ppost_gen(1), 0.5, 0),
                    (("pre", 2), lambda: ppre_gen(2), 0.6, 8)],
                2: [(("post", 2), lambda: ppost_gen(2), 0.5, 0),
                    (("pre", 3), lambda: ppre_gen(3), 0.5, 6),
                    (("out", 0), lambda: out_gen(0), 0.35, 16)],
                3: [(("post", 3), lambda: ppost_gen(3), 0.5, 0),
                    (("out", 0), lambda: out_gen(0), 0.5, 2),
                    (("out", 1), lambda: out_gen(1), 0.45, 4),
                    (("out", 2), lambda: out_gen(2), 0.45, 8)],
            }
            done = set()

            def pump(key, mk):
                g = gget(key, mk)
                if key in done:
                    return False
                try:
                    next(g)
                    return True
                except StopIteration:
                    done.add(key)
                    return False

            for qb in range(ST):
                # attention(qb) requires its projection prefix complete
                if qb > 0:
                    while pump(("pre", qb), lambda: ppre_gen(qb)):
                        pass
                fillers = [[key, mk, rate, delay, 0.0]
                           for (key, mk, rate, delay) in plans[qb]]
                # emit each pre-filler's first unit (its DMA block) up front
                for f in fillers:
                    if f[0] not in gens and f[0][0] == "pre":
                        pump(f[0], f[1])
                ui = 0
                for _ in attn_gen(qb):
                    ui += 1
                    for f in fillers:
                        if ui <= f[3]:
                            continue
                        f[4] += f[2]
                        while f[4] >= 1.0:
                            f[4] -= 1.0
                            if not pump(f[0], f[1]):
                                f[4] = -1e9
                                break
                for f in fillers:
                    if f[0][0] == "post":
                        while pump(f[0], f[1]):
                            pass
            for qo in range(ST - 1):
                while pump(("out", qo), lambda: out_gen(qo)):
                    pass
            drain(out_gen(ST - 1))
    nc.compile()
    return nc


def _mask_tri():
    from ml_dtypes import bfloat16
    kp = np.arange(P)[:, None]
    qf = np.arange(P)[None, :]
    return (qf >= kp).astype(bfloat16)


def kernel(x, W_qkv, b_qkv, W_out, b_out):
    from concourse.bass_utils import run_bass_kernel_spmd
    from ml_dtypes import bfloat16

    if "nc" not in _cache:
        _cache["nc"] = _build()
    nc = _cache["nc"]

    x = np.asarray(x, dtype=np.float32)
    W_qkv = np.asarray(W_qkv, dtype=np.float32)
    b_qkv = np.asarray(b_qkv, dtype=np.float32)
    W_out = np.asarray(W_out, dtype=np.float32)
    b_out = np.asarray(b_out, dtype=np.float32)

    mtri = _mask_tri()
    idn = np.eye(P, dtype=bfloat16)
    in_maps = []
    for c in range(NCORES):
        b, g = c % B, c // B
        hs = slice(g * HC, (g + 1) * HC)
        Wl = W_qkv[:, :, hs, :]                       # [E, 3, HC, D]
        in_maps.append({
            "xT": np.ascontiguousarray(x[b].T).astype(bfloat16),
            "wq": np.ascontiguousarray(
                Wl[:, 0].reshape(E, FQ)).astype(bfloat16),
            "wk": np.ascontiguousarray(
                Wl[:, 1].reshape(E, FQ)).astype(bfloat16),
            "wv": np.ascontiguousarray(
                Wl[:, 2].reshape(E, FQ)).astype(bfloat16),
            "wo": np.ascontiguousarray(
                W_out[hs].reshape(FQ, E)).astype(bfloat16),
            "msk": mtri,
            "idn": idn,
            "bq": np.ascontiguousarray(b_qkv[0, hs].reshape(FQ)),
            "bk": np.ascontiguousarray(b_qkv[1, hs].reshape(FQ)),
            "bvb": np.broadcast_to(b_qkv[2, hs].reshape(1, FQ),
                                   (P, FQ)).copy(),
        })

    try:
        res = run_bass_kernel_spmd(nc, in_maps, core_ids=list(range(NCORES)))
    except Exception:
        # transient device wedges (NRT_EXEC_UNIT_UNRECOVERABLE) clear on retry
        res = run_bass_kernel_spmd(nc, in_maps, core_ids=list(range(NCORES)))
    _cache["last_results"] = res
    out = np.empty((B, S, E), dtype=np.float32)
    for b in range(B):
        out[b] = (res.results[b]["outT"].T + res.results[b + B]["outT"].T
                  + b_out)
    return out


# revision 6
# speedup vs baseline: 1.0771x; 1.0771x over previous
"""Causal MHA block (B=4,S=2048,E=1024,H=16,D=64) on 8 trn2 cores — v4.

Sharding: 4 batches x 2 head-groups (8 heads each) = 8 cores; host sums the
2 partials per batch and adds b_out.

Layout (cost model charges a matmul out_free_size rows regardless of
contraction depth or output partition count):
  - AV in NATURAL layout: ans[q=128, 65] per (head, q-subtile); lhsT =
    exp'd scores w[k, q] (stationary), rhs = V[k, 65] (64 v-cols + ones col
    -> fused softmax denominator). Dead q-subtiles above the diagonal are
    skipped; diagonal subtile masked with a [128,128] triangle.
  - All inputs bf16 (1 cyc/row at ANY width -> exact causal widths); QKV
    projection matmuls bf16 with f32 psum accumulation.
  - ans normalized per q-subtile as soon as its last AV lands (reciprocal
    of the denom col + one strided stt-mul), PE-transposed (bf16 identity)
    into the ans psum banks (bitcast), copied to SBUF for the out proj.

Schedule: attention(qb) is ACT(exp)-bound and increasingly so for later
blocks; projections/out-projections are PE-only. Each s-block's projection
is split into a prefix (x/w loads, Q all, K ft0, V all) that gates
attention(qb), and a postfix (K ft1-3) consumed as filler inside
attention(qb). Out-projections are deferred into the latest (most
ACT-bound) blocks. Fillers are drip-fed with per-generator rate/delay.
"""

import numpy as np

B, S, E, H, D = 4, 2048, 1024, 16, 64
NCORES = 8
HG = 2                 # head groups (tensor parallel)
HC = H // HG           # 8 heads per core
FQ = HC * D            # 512 local features per q/k/v
P, NB = 128, 512       # partition tile, free-dim block
ET, ST, KTN, FT = E // P, S // NB, S // P, FQ // P   # 8, 4, 16, 4
EP = ET // 2           # e pair-tiles
DV = D + 1             # v cols + ones (denominator) col

_cache = {}


def _build():
    from contextlib import ExitStack
    import concourse.tile as tile
    import concourse.mybir as mybir
    from concourse import bacc

    dt = mybir.dt
    f32, bf16 = dt.float32, dt.bfloat16
    AF = mybir.ActivationFunctionType
    ALU = mybir.AluOpType
    SCALE = 0.125  # 1/sqrt(D)

    nc = bacc.Bacc("TRN2", target_bir_lowering=False, debug=False,
                   num_devices=NCORES)

    xT = nc.dram_tensor("xT", [E, S], bf16, kind="ExternalInput").ap()
    wq = nc.dram_tensor("wq", [E, FQ], bf16, kind="ExternalInput").ap()
    wk = nc.dram_tensor("wk", [E, FQ], bf16, kind="ExternalInput").ap()
    wv = nc.dram_tensor("wv", [E, FQ], bf16, kind="ExternalInput").ap()
    wo = nc.dram_tensor("wo", [FQ, E], bf16, kind="ExternalInput").ap()
    msk = nc.dram_tensor("msk", [P, P], bf16, kind="ExternalInput").ap()
    idn = nc.dram_tensor("idn", [P, P], bf16, kind="ExternalInput").ap()
    bq = nc.dram_tensor("bq", [FQ], f32, kind="ExternalInput").ap()
    bk = nc.dram_tensor("bk", [FQ], f32, kind="ExternalInput").ap()
    bvb = nc.dram_tensor("bvb", [P, FQ], f32, kind="ExternalInput").ap()
    outT = nc.dram_tensor("outT", [E, S], f32, kind="ExternalOutput").ap()

    with tile.TileContext(nc) as tc:
        with ExitStack() as ctx:
            pers = ctx.enter_context(tc.tile_pool(name="pers", bufs=1))
            pmisc = ctx.enter_context(tc.tile_pool(name="pmisc", bufs=1))
            px = ctx.enter_context(tc.tile_pool(name="px", bufs=2))
            pwk = ctx.enter_context(tc.tile_pool(name="pwk", bufs=2))
            pwv = ctx.enter_context(tc.tile_pool(name="pwv", bufs=1))
            pwq = ctx.enter_context(tc.tile_pool(name="pwq", bufs=1))
            pqts = ctx.enter_context(tc.tile_pool(name="pqts", bufs=2))
            pwe = ctx.enter_context(tc.tile_pool(name="pwe", bufs=5))
            pans = ctx.enter_context(tc.tile_pool(name="pans", bufs=2))
            patt = ctx.enter_context(tc.tile_pool(name="patt", bufs=4))
            pepi = ctx.enter_context(tc.tile_pool(name="pepi", bufs=3))
            pout = ctx.enter_context(tc.tile_pool(name="pout", bufs=4))
            ps1 = ctx.enter_context(
                tc.tile_pool(name="ps1", bufs=2, space="PSUM"))
            sps = ctx.enter_context(
                tc.tile_pool(name="sps", bufs=2, space="PSUM"))
            avps = ctx.enter_context(
                tc.tile_pool(name="avps", bufs=1, space="PSUM"))

            KT = [pers.tile([P, S], bf16, tag=f"kt{i}", name=f"kt{i}")
                  for i in range(FT)]
            Vp = [pers.tile([P, HC * DV], bf16, tag=f"vp{i}",
                            name=f"vp{i}") for i in range(KTN)]
            bqt = pers.tile([P, FT], f32, tag="bqt")
            bkt = pers.tile([P, FT], f32, tag="bkt")
            bvt = pers.tile([P, FQ], f32, tag="bvt")

            mtri = pmisc.tile([P, P], bf16, tag="mtri")
            ident = pmisc.tile([P, P], bf16, tag="ident")
            wouts = [pmisc.tile([P, E], bf16, tag=f"wo{ft}", name=f"wo{ft}")
                     for ft in range(FT)]

            def late_loads_gen():
                """Non-critical loads kept off the startup critical path."""
                nc.gpsimd.dma_start(bqt[:], bq.rearrange("(a p) -> p a", p=P))
                nc.gpsimd.dma_start(bkt[:], bk.rearrange("(a p) -> p a", p=P))
                nc.gpsimd.dma_start(bvt[:], bvb[:])
                nc.gpsimd.dma_start(mtri[:], msk[:])
                nc.gpsimd.dma_start(ident[:], idn[:])
                yield

            def wout_gen():
                """wout loads; needed only by OUT(0), driven as a filler so
                they stay off the startup critical path."""
                for ft in range(FT):
                    eng = nc.scalar if ft % 2 else nc.sync
                    eng.dma_start(wouts[ft][:],
                                  wo[ft * P:(ft + 1) * P, :])
                    yield

            # per-block state shared between generators
            QTS = {}    # sb -> [4 tiles]
            XTS = {}    # sb -> [4 pair tiles [P, 2, NB]]
            KWT = {}    # sb -> [4 wk pair tiles]
            VWT = {}    # sb -> [4 wv pair tiles]
            ATS = {}    # qb -> at tile [P, 4*NB] bf16 (q-part, qsub, hd)
            ATT = {}    # qb -> [4 atT tiles [P, NB] bf16] (hd-part, q)
            KPROG = {}  # sb -> set of K ft segments emitted
            VPROG = {}  # sb -> set of V stl tiles emitted
            NEED = {}   # callbacks set by the driver
            WQ = []     # resident wq pair tiles

            def ppre_gen(sb):
                """Q-prefix of s-block sb: x/w loads + Q pass (all ft).
                Gates attention(sb); K/V race the off-diagonal steps as
                kv_gen. The first x/wq pair is split per-e so the very
                first matmul starts sooner."""
                xts = []
                for ep in range(EP):
                    t = px.tile([P, 2, NB], bf16, tag=f"x{ep}",
                                name=f"x{ep}_{sb}")
                    if sb == 0 and ep == 0:
                        for a in range(2):
                            nc.sync.dma_start(
                                t[:, a, :],
                                xT[a * P:(a + 1) * P, 0:NB])
                    else:
                        nc.sync.dma_start(
                            t[:], xT[2 * ep * P:(2 * ep + 2) * P,
                                     sb * NB:(sb + 1) * NB]
                            .rearrange("(a p) s -> p a s", p=P))
                    xts.append(t)
                    if sb == 0:
                        wt = pwq.tile([P, 2, FQ], bf16, tag=f"wq{ep}",
                                      name=f"wq{ep}")
                        WQ.append(wt)
                        # interleave wq with x so pairs arrive together
                        if ep == 0:
                            for a in range(2):
                                nc.scalar.dma_start(
                                    wt[:, a, :], wq[a * P:(a + 1) * P, :])
                        else:
                            nc.scalar.dma_start(
                                wt[:], wq[2 * ep * P:(2 * ep + 2) * P, :]
                                .rearrange("(a p) f -> p a f", p=P))
                XTS[sb] = xts
                kwts = []
                for ep in range(EP):
                    t = pwk.tile([P, 2, FQ], bf16, tag=f"wk{ep}",
                                name=f"wk{ep}_{sb}")
                    nc.scalar.dma_start(
                        t[:], wk[2 * ep * P:(2 * ep + 2) * P, :]
                        .rearrange("(a p) f -> p a f", p=P))
                    kwts.append(t)
                KWT[sb] = kwts
                vwts = []
                for ep in range(EP):
                    t = pwv.tile([P, 2, FQ], bf16, tag=f"wv{ep}",
                                name=f"wv{ep}_{sb}")
                    nc.sync.dma_start(
                        t[:], wv[2 * ep * P:(2 * ep + 2) * P, :]
                        .rearrange("(a p) f -> p a f", p=P))
                    vwts.append(t)
                VWT[sb] = vwts
                yield
                # Q pass (all ft)
                QTS[sb] = []
                for ft in range(FT):
                    ps = ps1.tile([P, NB], f32, tag="ps", name=f"psq{ft}_{sb}")
                    for e in range(ET):
                        nc.tensor.matmul(
                            ps[:], WQ[e // 2][:, e % 2, ft * P:(ft + 1) * P],
                            xts[e // 2][:, e % 2, :], start=(e == 0),
                            stop=(e == ET - 1))
                        if e % 2 == 1 and e < ET - 1:
                            yield
                    qt = pqts.tile([P, NB], bf16, tag=f"qts{ft}",
                                   name=f"qts{ft}_{sb}")
                    nc.vector.tensor_scalar_add(qt[:], ps[:],
                                                bqt[:, ft:ft + 1])
                    QTS[sb].append(qt)
                    yield

            def kv_gen(sb):
                """K (ft0) + V (all stl): the highest-priority filler at the
                start of attention(sb) — K ft0 is needed at hp0's diagonal
                (step 4*sb), V at the diagonal AVs just after."""
                yield from kpass(sb, 0)
                KPROG.setdefault(sb, set()).add(0)
                xts, vwts = XTS[sb], VWT[sb]
                for stl in range(ST):
                    st = ST * sb + stl
                    ps = ps1.tile([P, NB], f32, tag="ps",
                                  name=f"psv{stl}_{sb}")
                    for e in range(ET):
                        nc.tensor.matmul(
                            ps[:], xts[e // 2][:, e % 2, stl * P:(stl + 1) * P],
                            vwts[e // 2][:, e % 2, :], start=(e == 0),
                            stop=(e == ET - 1))
                        if e % 2 == 1 and e < ET - 1:
                            yield
                    vview = Vp[st][:].rearrange("p (h c) -> p h c", c=DV)
                    nc.vector.memset(vview[:, :, D:DV], 1.0)
                    nc.vector.scalar_tensor_tensor(
                        vview[:, :, 0:D], ps[:], 1.0,
                        bvt[:].rearrange("p (h d) -> p h d", d=D),
                        op0=ALU.mult, op1=ALU.add)
                    yield

            def kpass(sb, ft):
                ps = ps1.tile([P, NB], f32, tag="ps", name=f"psk{ft}_{sb}")
                for e in range(ET):
                    nc.tensor.matmul(
                        ps[:], KWT[sb][e // 2][:, e % 2, ft * P:(ft + 1) * P],
                        XTS[sb][e // 2][:, e % 2, :], start=(e == 0),
                        stop=(e == ET - 1))
                    if e % 2 == 1 and e < ET - 1:
                        yield
                nc.vector.tensor_scalar_add(
                    KT[ft][:, sb * NB:(sb + 1) * NB], ps[:],
                    bkt[:, ft:ft + 1])
                yield

            def ppost_gen(sb):
                """Projection postfix: K ft1-3, consumed inside
                attention(sb) before each hp reaches its diagonal."""
                for ft in range(1, FT):
                    yield from kpass(sb, ft)
                    KPROG.setdefault(sb, set()).add(ft)

            def attn_gen(qb):
                """Attention for q-block qb. Yields once per kt step.
                The per-qsub epilogue (normalize) fires as soon as that
                q-subtile's last AV lands; each head-pair's at columns are
                PE-transposed right after its last epilogue."""
                nkt = ST * (qb + 1)
                QTs = QTS[qb]
                at = pans.tile([P, ST * NB], bf16, tag="at", name=f"at{qb}")
                ATS[qb] = at
                ATT[qb] = [None] * FT
                atv = at[:].rearrange("p (g f) -> p g f", f=NB)
                for hp in range(FT):
                    # ans psum: per head i: [P, 4*DV] = 4 q-subtiles x
                    # (64 v + denom); accumulation groups per slice.
                    ans = [avps.tile([P, ST * DV], f32, tag=f"an{i}",
                                     name=f"an{qb}_{hp}_{i}")
                           for i in range(2)]
                    anvs = [a[:].rearrange("p (g c) -> p g c", c=DV)
                            for a in ans]
                    recs = [pepi.tile([P, ST], f32, tag=f"rec{i}",
                                      name=f"rec{qb}_{hp}_{i}")
                            for i in range(2)]

                    def emit_epi(g):
                        # normalize q-subtile g of both heads: its last AV
                        # (kt = ST*qb + g) has landed.
                        for i in range(2):
                            nc.vector.reciprocal(
                                recs[i][:, g:g + 1]
                                .rearrange("p (a c) -> p a c", c=1),
                                anvs[i][:, g:g + 1, D:DV])
                            nc.vector.tensor_scalar_mul(
                                atv[:, g:g + 1,
                                    (2 * hp + i) * D:(2 * hp + i + 1) * D],
                                anvs[i][:, g:g + 1, 0:D],
                                recs[i][:, g:g + 1])

                    def emit_av(ent, last):
                        kt0, j0, w0 = ent
                        if j0 >= 0:
                            # V tile for the diagonal must be emitted first
                            NEED["v"](qb, j0)
                        wvw = w0[:].rearrange("p (h q) -> p h q", h=2)
                        for i in range(2):
                            rv = Vp[kt0][:, (2 * hp + i) * DV:
                                         (2 * hp + i + 1) * DV]
                            for g in range(max(j0, 0), ST):
                                # start=True lazily zeroes the WHOLE 2KB
                                # psum bank (ZERO_REGION), so only the first
                                # matmul into the bank may set it; later
                                # slices accumulate onto the zeroed region.
                                nc.tensor.matmul(
                                    ans[i][:, g * DV:(g + 1) * DV],
                                    wvw[:, i, g * P:(g + 1) * P],
                                    rv,
                                    start=(kt0 == 0 and g == max(j0, 0)),
                                    stop=(kt0 == ST * qb + g),
                                    skip_group_check=True)
                        if j0 >= 0:
                            emit_epi(j0)

                    pend = []
                    for kt in range(nkt):
                        j = kt - ST * qb
                        c0 = j * P if j >= 0 else 0
                        if j == 0:
                            # structural gate: K(ft=hp) segment qb must be
                            # emitted before its diagonal scores read it
                            NEED["k"](qb, hp)
                        # both heads of the pair share one 2-bank psum tile
                        # and a single strided exp call
                        sp = sps.tile([P, 2 * NB], f32, tag="sp",
                                      name=f"sp{qb}_{hp}_{kt}")
                        for i in range(2):
                            nc.tensor.matmul(
                                sp[:, i * NB + c0:(i + 1) * NB],
                                KT[hp][i * D:(i + 1) * D,
                                       kt * P:(kt + 1) * P],
                                QTs[hp][i * D:(i + 1) * D, c0:NB],
                                start=True, stop=True)
                        w = pwe.tile([P, 2 * NB], bf16, tag="w",
                                     name=f"w{qb}_{hp}_{kt}")
                        spv = sp[:].rearrange("p (h q) -> p h q", h=2)
                        wv_ = w[:].rearrange("p (h q) -> p h q", h=2)
                        nc.scalar.activation(wv_[:, :, c0:NB],
                                             spv[:, :, c0:NB],
                                             AF.Exp, scale=SCALE)
                        if j >= 0:
                            nc.vector.tensor_mul(
                                wv_[:, :, c0:c0 + P], wv_[:, :, c0:c0 + P],
                                mtri[:]
                                .rearrange("p (a q) -> p a q", a=1)
                                .broadcast_to([P, 2, P]))
                        pend.append((kt, j, w))
                        if len(pend) > 2:
                            emit_av(pend.pop(0), last=False)
                        yield
                    while pend:
                        ent = pend.pop(0)
                        emit_av(ent, last=not pend)
                        yield
                    # transpose this head-pair's at columns into atT
                    # (the ans psum banks are free again; bitcast bf16)
                    tpf = avps.tile([P, ST * DV], f32, tag=f"an{hp % 2}",
                                    name=f"tp{qb}_{hp}")
                    tp = tpf[:].bitcast(bf16)
                    for g in range(ST):
                        # one bank-zeroing group across the 4 block writes
                        nc.tensor.matmul(
                            tp[:, g * P:(g + 1) * P],
                            atv[:, g, hp * P:(hp + 1) * P],
                            ident[:], is_transpose=True,
                            start=(g == 0), stop=(g == ST - 1),
                            skip_group_check=True)
                    yield
                    att = patt.tile([P, NB], bf16, tag=f"att{hp}",
                                    name=f"att{qb}_{hp}")
                    nc.vector.tensor_copy(att[:], tp[:, 0:NB])
                    ATT[qb][hp] = att
                    yield

            def out_gen(qb):
                """Output projection of q-block qb. Yields per e-tile."""
                for et in range(ET):
                    pot = ps1.tile([P, NB], f32, tag="ps",
                                   name=f"po{qb}_{et}")
                    po = pot[:]
                    for ft in range(FT):
                        nc.tensor.matmul(po,
                                         wouts[ft][:, et * P:(et + 1) * P],
                                         ATT[qb][ft][:], start=(ft == 0),
                                         stop=(ft == FT - 1))
                        if ft == 1:
                            yield
                    ott = pout.tile([P, NB], f32, tag="ot",
                                    name=f"ot{qb}_{et}")
                    if qb == ST - 1 and et % 2:
                        nc.scalar.copy(ott[:], po)
                    else:
                        nc.vector.tensor_copy(ott[:], po)
                    if qb == ST - 1:
                        eng = (nc.sync, nc.scalar)[et % 2]
                    else:
                        eng = nc.gpsimd
                    eng.dma_start(
                        outT[et * P:(et + 1) * P, qb * NB:(qb + 1) * NB],
                        ott[:])
                    yield

            def drain(g):
                for _ in g:
                    pass

            gens = {}

            def gget(key, mk):
                if key not in gens:
                    gens[key] = mk()
                return gens[key]

            p0 = gget(("pre", 0), lambda: ppre_gen(0))
            next(p0)          # x/w DMAs emitted first
            drain(late_loads_gen())
            drain(p0)
            # Filler plan per attention block: (key, maker, rate, delay).
            # Rates in filler-units per attention kt-step; delay in steps.
            # kv(qb) is the critical filler: raced at high rate from step 0
            # against hp0's diagonal deadline.
            plans = {
                0: [(("post", 0), lambda: ppost_gen(0), 0.9, 0),
                    (("wout",), wout_gen, 0.3, 2),
                    (("pre", 1), lambda: ppre_gen(1), 1.5, 10)],
                1: [(("kv", 1), lambda: kv_gen(1), 3.0, 0),
                    (("post", 1), lambda: ppost_gen(1), 0.3, 10),
                    (("pre", 2), lambda: ppre_gen(2), 0.65, 22)],
                2: [(("kv", 2), lambda: kv_gen(2), 3.0, 0),
                    (("post", 2), lambda: ppost_gen(2), 0.25, 14),
                    (("pre", 3), lambda: ppre_gen(3), 0.55, 24),
                    (("out", 0), lambda: out_gen(0), 0.45, 42)],
                3: [(("kv", 3), lambda: kv_gen(3), 3.0, 0),
                    (("post", 3), lambda: ppost_gen(3), 0.2, 12),
                    (("out", 0), lambda: out_gen(0), 0.35, 10),
                    (("out", 1), lambda: out_gen(1), 0.32, 24),
                    (("out", 2), lambda: out_gen(2), 0.32, 38)],
            }
            done = set()

            def pump(key, mk):
                g = gget(key, mk)
                if key in done:
                    return False
                try:
                    next(g)
                    return True
                except StopIteration:
                    done.add(key)
                    return False

            def need_k(qb, hp):
                if hp == 0:
                    while 0 not in KPROG.get(qb, set()) and \
                            pump(("kv", qb), lambda: kv_gen(qb)):
                        pass
                else:
                    while hp not in KPROG.get(qb, set()) and \
                            pump(("post", qb), lambda: ppost_gen(qb)):
                        pass

            def need_v(qb, stl):
                while stl not in VPROG.get(qb, set()) and \
                        pump(("kv", qb), lambda: kv_gen(qb)):
                    pass

            NEED["k"] = need_k
            NEED["v"] = need_v

            # attention(0)'s kt0 is diagonal: K0+V(0) must be emitted now
            while pump(("kv", 0), lambda: kv_gen(0)):
                pass

            for qb in range(ST):
                # attention(qb) requires its Q prefix complete
                if qb > 0:
                    while pump(("pre", qb), lambda: ppre_gen(qb)):
                        pass
                fillers = [[key, mk, rate, delay, 0.0]
                           for (key, mk, rate, delay) in plans[qb]]
                # emit each pre-filler's first unit (its DMA block) up front
                for f in fillers:
                    if f[0] not in gens and f[0][0] == "pre":
                        pump(f[0], f[1])
                ui = 0
                for _ in attn_gen(qb):
                    ui += 1
                    for f in fillers:
                        if ui <= f[3]:
                            continue
                        f[4] += f[2]
                        while f[4] >= 1.0:
                            f[4] -= 1.0
                            if not pump(f[0], f[1]):
                                f[4] = -1e9
                                break
                for f in fillers:
                    if f[0][0] in ("post", "kv"):
                        while pump(f[0], f[1]):
                            pass
            for qo in range(ST - 1):
                while pump(("out", qo), lambda: out_gen(qo)):
                    pass
            drain(out_gen(ST - 1))
    nc.compile()
    return nc


def _mask_tri():
    from ml_dtypes import bfloat16
    kp = np.arange(P)[:, None]
    qf = np.arange(P)[None, :]
    return (qf >= kp).astype(bfloat16)


def kernel(x, W_qkv, b_qkv, W_out, b_out):
    from concourse.bass_utils import run_bass_kernel_spmd
    from ml_dtypes import bfloat16

    if "nc" not in _cache:
        _cache["nc"] = _build()
    nc = _cache["nc"]

    x = np.asarray(x, dtype=np.float32)
    W_qkv = np.asarray(W_qkv, dtype=np.float32)
    b_qkv = np.asarray(b_qkv, dtype=np.float32)
    W_out = np.asarray(W_out, dtype=np.float32)
    b_out = np.asarray(b_out, dtype=np.float32)

    mtri = _mask_tri()
    idn = np.eye(P, dtype=bfloat16)
    in_maps = []
    for c in range(NCORES):
        b, g = c % B, c // B
        hs = slice(g * HC, (g + 1) * HC)
        Wl = W_qkv[:, :, hs, :]                       # [E, 3, HC, D]
        in_maps.append({
            "xT": np.ascontiguousarray(x[b].T).astype(bfloat16),
            "wq": np.ascontiguousarray(
                Wl[:, 0].reshape(E, FQ)).astype(bfloat16),
            "wk": np.ascontiguousarray(
                Wl[:, 1].reshape(E, FQ)).astype(bfloat16),
            "wv": np.ascontiguousarray(
                Wl[:, 2].reshape(E, FQ)).astype(bfloat16),
            "wo": np.ascontiguousarray(
                W_out[hs].reshape(FQ, E)).astype(bfloat16),
            "msk": mtri,
            "idn": idn,
            "bq": np.ascontiguousarray(b_qkv[0, hs].reshape(FQ)),
            "bk": np.ascontiguousarray(b_qkv[1, hs].reshape(FQ)),
            "bvb": np.broadcast_to(b_qkv[2, hs].reshape(1, FQ),
                                   (P, FQ)).copy(),
        })

    try:
        res = run_bass_kernel_spmd(nc, in_maps, core_ids=list(range(NCORES)))
    except Exception:
        # transient device wedges (NRT_EXEC_UNIT_UNRECOVERABLE) clear on retry
        res = run_bass_kernel_spmd(nc, in_maps, core_ids=list(range(NCORES)))
    _cache["last_results"] = res
    out = np.empty((B, S, E), dtype=np.float32)
    for b in range(B):
        out[b] = (res.results[b]["outT"].T + res.results[b + B]["outT"].T
                  + b_out)
    return out
